# revision 1
# baseline (speedup 1.0000x reference)
"""PolyLoRALinear TRN2 kernel.

Math (per batch example b, with n_splits=1):
  mw[b]   = normalize(sigmoid(module_logits[task_ids[b]]))          # [8]
  A_b     = sum_s mw[b,s] * lora_a[0,s]                             # [4096, 16]
  B_b     = sum_s mw[b,s] * lora_b[0,s] / rank                      # [16, 4096]
  out[b]  = x[b] @ W.T + bias + (x[b] @ A_b) @ B_b

Sharding: 2 (batch groups of 4 examples) x 4 (out-feature shards of 1024)
mesh over 8 cores. Weight/lora mixing + all layout transposes happen on
host; the device kernel is a pure fp32r GEMM with the rank-16 adapter and
bias fused into the PSUM accumulation:

  per 128-token tile t, per 512-wide out block:
    psum  = ones.T @ bias_blk          (K=1 matmul, start=True)
    psum += sum_i xT_i.T @ WT_i_blk    (32 K=128 matmuls)
    psum += xaT.T @ B_blk              (K=16 matmul; xaT from on-chip
                                        xa = x@A + PE transpose)
"""

import sys

import numpy as np

sys.path.insert(0, "/opt/trn_rl_repo")

import concourse.bacc as bacc
import concourse.mybir as mybir
import concourse.tile as tile
from concourse import bass_utils
from concourse.bass import ts

F32 = mybir.dt.float32
F32R = mybir.dt.float32r

BS, SEQ, D_IN, D_OUT = 8, 2048, 4096, 4096
N_TASKS, N_SKILLS, RANK = 256, 8, 16
EPS = 1e-12

GB, GO = 2, 4  # mesh: batch groups x out-feature shards
EX = BS // GB  # examples per core (4)
TOK = EX * SEQ  # tokens per core (8192)
OSH = D_OUT // GO  # out features per core (1024)
TT = TOK // 128  # 128-token tiles per core (64)
KI = D_IN // 128  # contraction subtiles (32)
NB = OSH // 512  # 512-wide out blocks per core (2)
T_PER_EX = SEQ // 128  # token tiles per example (16)

_CACHE = {}


def _build():
    if "nc" in _CACHE:
        return _CACHE["nc"]

    nc = bacc.Bacc("TRN2", target_bir_lowering=False, debug=False)
    xt_d = nc.dram_tensor("xt", [TT, 128, D_IN], F32R, kind="ExternalInput").ap()
    wt_d = nc.dram_tensor("wt", [128, KI * OSH], F32R, kind="ExternalInput").ap()
    a_d = nc.dram_tensor("a", [128, EX * KI * RANK], F32R, kind="ExternalInput").ap()
    b_d = nc.dram_tensor("b", [RANK, EX * OSH], F32R, kind="ExternalInput").ap()
    bias_d = nc.dram_tensor("bias", [1, OSH], F32R, kind="ExternalInput").ap()
    ones_d = nc.dram_tensor("ones", [1, 128], F32R, kind="ExternalInput").ap()
    ident_d = nc.dram_tensor("ident", [128, 128], F32R, kind="ExternalInput").ap()
    out_d = nc.dram_tensor("out", [TT, 128, OSH], F32, kind="ExternalOutput").ap()

    with tile.TileContext(nc) as tc:
        with (
            tc.tile_pool(name="const", bufs=1) as cst,
            tc.tile_pool(name="xt", bufs=2) as xtp,
            tc.tile_pool(name="osb", bufs=4) as osb,
            tc.tile_pool(name="small", bufs=2) as small,
            tc.tile_pool(name="ps_out", bufs=4, space="PSUM") as ps_out,
            tc.tile_pool(name="ps_xa", bufs=2, space="PSUM") as ps_xa,
        ):
            wt_sb = cst.tile([128, KI * OSH], F32R)
            nc.sync.dma_start(wt_sb[:], wt_d[:])
            a_sb = cst.tile([128, EX * KI * RANK], F32R)
            nc.sync.dma_start(a_sb[:], a_d[:])
            b_sb = cst.tile([RANK, EX * OSH], F32R)
            nc.sync.dma_start(b_sb[:], b_d[:])
            bias_sb = cst.tile([1, OSH], F32R)
            nc.sync.dma_start(bias_sb[:], bias_d[:])
            ones = cst.tile([1, 128], F32R)
            nc.sync.dma_start(ones[:], ones_d[:])
            ident = cst.tile([128, 128], F32R)
            nc.sync.dma_start(ident[:], ident_d[:])

            for tt in range(TT):
                ex = tt // T_PER_EX
                xt_sb = xtp.tile([128, D_IN], F32R)
                nc.sync.dma_start(xt_sb[:], xt_d[tt])

                # xa[t, r] = sum_i x[t, i] * A[i, r]
                psum_xa = ps_xa.tile([128, RANK], F32, name="xa")
                for i in range(KI):
                    nc.tensor.matmul(
                        psum_xa[:],
                        xt_sb[:, ts(i, 128)],
                        a_sb[:, ts(ex * KI + i, RANK)],
                        start=(i == 0),
                        stop=(i == KI - 1),
                    )
                xa_sb = small.tile([128, RANK], F32R, name="xa_sb")
                nc.vector.tensor_copy(xa_sb[:], psum_xa[:])
                psum_xat = ps_xa.tile([RANK, 128], F32R, name="xat")
                nc.tensor.transpose(psum_xat[:], xa_sb[:], ident[:])
                xat_sb = small.tile([RANK, 128], F32R, name="xat_sb")
                nc.vector.tensor_copy(xat_sb[:], psum_xat[:])

                for ob in range(NB):
                    psum = ps_out.tile([128, 512], F32, name="acc")
                    nc.tensor.matmul(
                        psum[:],
                        ones[:],
                        bias_sb[:, ts(ob, 512)],
                        start=True,
                        stop=False,
                    )
                    for i in range(KI):
                        nc.tensor.matmul(
                            psum[:],
                            xt_sb[:, ts(i, 128)],
                            wt_sb[:, i * OSH + ob * 512 : i * OSH + ob * 512 + 512],
                            start=False,
                            stop=False,
                        )
                    nc.tensor.matmul(
                        psum[:],
                        xat_sb[:],
                        b_sb[:, ex * OSH + ob * 512 : ex * OSH + ob * 512 + 512],
                        start=False,
                        stop=True,
                    )
                    out_sb = osb.tile([128, 512], F32, name="out_sb")
                    nc.scalar.copy(out_sb[:], psum[:])
                    nc.sync.dma_start(out_d[tt][:, ts(ob, 512)], out_sb[:])

    nc.compile()
    _CACHE["nc"] = nc
    return nc


def _host_prep(x, task_ids, weight, bias, module_logits, lora_a, lora_b):
    x = np.asarray(x, dtype=np.float32)
    task_ids = np.asarray(task_ids).astype(np.int64)
    weight = np.asarray(weight, dtype=np.float32)
    bias = np.asarray(bias, dtype=np.float32)
    module_logits = np.asarray(module_logits, dtype=np.float32)
    lora_a = np.asarray(lora_a, dtype=np.float32)
    lora_b = np.asarray(lora_b, dtype=np.float32)

    # routing weights (PolytroponSelector eval path)
    logits = module_logits[task_ids].reshape(BS, N_SKILLS)
    mw = 1.0 / (1.0 + np.exp(-logits))
    mw = mw / (mw.sum(axis=-1, keepdims=True) + EPS)  # [8, 8]

    # mixed per-example LoRA mats
    a_mix = np.einsum("bs,sdr->bdr", mw, lora_a[0])  # [8, 4096, 16]
    b_mix = np.einsum("bs,srd->brd", mw, lora_b[0]) / RANK  # [8, 16, 4096]

    ones = np.ones((1, 128), dtype=np.float32)
    ident = np.eye(128, dtype=np.float32)

    # per-batch-group tensors (shared by the 4 o-shard cores of a group)
    xt_g, a_g = [], []
    for g in range(GB):
        xf = x[g * EX : (g + 1) * EX].reshape(TOK, D_IN)
        # xt[tt, p, i*128+f] = x[tt*128+f, i*128+p]
        xt = np.ascontiguousarray(
            xf.reshape(TT, 128, KI, 128).transpose(0, 3, 2, 1)
        ).reshape(TT, 128, D_IN)
        xt_g.append(xt)
        # a[p, ((ex*KI)+i)*16+r] = a_mix[g*EX+ex, i*128+p, r]
        am = a_mix[g * EX : (g + 1) * EX].reshape(EX, KI, 128, RANK)
        a_g.append(np.ascontiguousarray(am.transpose(2, 0, 1, 3)).reshape(128, -1))

    # per-o-shard tensors (shared by the 2 batch-group cores of a shard)
    wt_j, bias_j = [], []
    for j in range(GO):
        ws = weight[j * OSH : (j + 1) * OSH]  # [1024, 4096]
        wt = np.ascontiguousarray(
            ws.reshape(OSH, KI, 128).transpose(2, 1, 0)
        ).reshape(128, KI * OSH)
        wt_j.append(wt)
        bias_j.append(np.ascontiguousarray(bias[j * OSH : (j + 1) * OSH])[None, :])

    in_maps = []
    for c in range(8):
        g, j = c // GO, c % GO
        bm = b_mix[g * EX : (g + 1) * EX, :, j * OSH : (j + 1) * OSH]  # [4, 16, 1024]
        b_in = np.ascontiguousarray(bm.transpose(1, 0, 2)).reshape(RANK, EX * OSH)
        in_maps.append(
            {
                "xt": xt_g[g],
                "wt": wt_j[j],
                "a": a_g[g],
                "b": b_in,
                "bias": bias_j[j],
                "ones": ones,
                "ident": ident,
            }
        )
    return in_maps


def _gather(results):
    out = np.empty((GB, TOK, D_OUT), dtype=np.float32)
    for c in range(8):
        g, j = c // GO, c % GO
        out[g, :, j * OSH : (j + 1) * OSH] = results[c]["out"].reshape(TOK, OSH)
    return out.reshape(BS, SEQ, D_OUT)


def kernel(x, task_ids, weight, bias, module_logits, lora_a, lora_b, _trace=False):
    nc = _build()
    in_maps = _host_prep(x, task_ids, weight, bias, module_logits, lora_a, lora_b)
    res = bass_utils.run_bass_kernel_spmd(
        nc, in_maps, list(range(8)), trace=_trace
    )
    out = _gather(res.results)
    if _trace:
        return out, res
    return out
